# revision 102
# baseline (speedup 1.0000x reference)
"""AttentionSubsample Trainium2 kernel.

Full (unsharded) inputs in, full output out. Data-parallel over batch:
32 batches -> 8 NeuronCores x 4 batches each. Weights/biases replicated.

Per-core dataflow (per batch element), all matmuls bf16 (fp32 PSUM accum):
  kv proj   : kT[d,n] per head-pair + v[n,d] (2-bank psum tiles, one big
              psum->sbuf copy each, split ACT/DVE)
  q proj    : qT[d,q] per head-pair (BN bias fused into the ACT psum->sbuf
              copy; attention scale folded into W_q on host)
  per head  : scoresT[n,q] = kT.T @ qT (PE, K=64) in psum groups of 2
              n-chunks; e = exp(scores) (ACT) -> bf16 sbuf;
              e *= exp(bias) (DVE 2x-mode bf16 mult; exp(bias) is
              precomputed on host and resident in SBUF all run);
              oT[d,q] += v.T @ e (PE);
              softmax denominators: middle chunk-pairs are pre-added on the
              DVE (halves the PE ones-matmul cost), while the first/last
              groups go direct to the PE so the accumulation start isn't
              gated on the DVE and recip can start immediately at head end
  normalize : oT = po * recip(sums) read straight from PSUM (one DVE mult,
              no intermediate copy); hswish on the otherwise-idle Pool
              engine (1/6 folded into W_p on host)
  proj      : out[q,384] = h.T @ WpT + bp per batch PAIR (q=640 = 5x128)
Key algebraic facts used:
  - the k-projection BN bias adds a per-query-column constant to scores,
    which softmax is invariant to -> dropped entirely.
  - the v-channel BN bias folds out: softmax rows sum to 1, so
    attn@(v+bv) = attn@v + bv, applied per-partition after normalization.
  - exp(s+b) = exp(s)*exp(b): the relative-position bias becomes a bf16
    SBUF multiply on the DVE (2x mode) instead of an f32 PSUM add.
Startup is DMA-paced: wk is split into two tiles and batch 0's first x
columns arrive in 256-wide pieces paired with 256-wide first matmuls.
fp8 was evaluated and rejected: the attention here is sharp (scores up to
~9), so per-element e/v quantization errors do not average out (measured
1.7e-2..8e-2 rel err vs the 2e-2 gate).
"""

import sys

if "/opt/trn_rl_repo" not in sys.path:
    sys.path.insert(0, "/opt/trn_rl_repo")

import ml_dtypes
import numpy as np

# --- problem constants (hardcoded, must match the grading reference) ---
B, N, C = 32, 1280, 256
H, KD, D = 8, 64, 128          # heads, key dim, value dim per head
NQ = 320                       # subsampled sequence length
OUT = 384
NCORES = 8
BPC = B // NCORES              # batches per core
EPS = 1e-5
NCH = N // 128                 # 10 n-chunks of 128
GRP = 2                        # scores psum group size (n-chunks per group)

_SUB_IDX = np.concatenate([
    (np.arange(32)[::2][:, None] * 32 + np.arange(32)[::2][None, :]).reshape(-1),
    1024 + (np.arange(16)[::2][:, None] * 16 + np.arange(16)[::2][None, :]).reshape(-1),
])  # [320] subsample row gather


def _prep(inputs):
    """Host-side: fold BN into weights, reorder channels, shard over cores."""
    f32 = np.float32
    x = np.asarray(inputs["x"], f32)
    g_kv, b_kv = np.asarray(inputs["g_kv"], f32), np.asarray(inputs["b_kv"], f32)
    rm_kv, rv_kv = np.asarray(inputs["rm_kv"], f32), np.asarray(inputs["rv_kv"], f32)
    g_q, b_q = np.asarray(inputs["g_q"], f32), np.asarray(inputs["b_q"], f32)
    rm_q, rv_q = np.asarray(inputs["rm_q"], f32), np.asarray(inputs["rv_q"], f32)
    g_p, b_p = np.asarray(inputs["g_p"], f32), np.asarray(inputs["b_p"], f32)
    rm_p, rv_p = np.asarray(inputs["rm_p"], f32), np.asarray(inputs["rv_p"], f32)
    W_kv = np.asarray(inputs["W_kv"], f32)
    W_q = np.asarray(inputs["W_q"], f32)
    W_p = np.asarray(inputs["W_p"], f32)
    attn_bias = np.asarray(inputs["attn_bias"], f32)
    bias_idxs = np.asarray(inputs["bias_idxs"])

    s_kv = g_kv / np.sqrt(rv_kv + EPS)
    Wkv_f = W_kv * s_kv[:, None]
    bkv_f = b_kv - rm_kv * s_kv
    kidx = np.concatenate([np.arange(h * 192, h * 192 + KD) for h in range(H)])
    vidx = np.concatenate([np.arange(h * 192 + KD, (h + 1) * 192) for h in range(H)])
    bf = ml_dtypes.bfloat16
    wkt = np.ascontiguousarray(Wkv_f[kidx].T).reshape(2, 128, 512).astype(bf)
    wvt = np.ascontiguousarray(Wkv_f[vidx].T).reshape(2, 128, 1024).astype(bf)
    # k BN bias dropped: adds a per-q-column constant to scores (softmax-invariant)
    bvd = np.ascontiguousarray(bkv_f[vidx].reshape(8, 128).T)          # [128, H]

    scale = KD ** -0.5
    s_q = g_q / np.sqrt(rv_q + EPS)
    wqt = np.ascontiguousarray((W_q * (s_q * scale)[:, None]).T).reshape(2, 128, 512).astype(bf)
    bq = np.ascontiguousarray(((b_q - rm_q * s_q) * scale).reshape(4, 128).T)

    s_p = g_p / np.sqrt(rv_p + EPS)
    wpt = np.ascontiguousarray((W_p * s_p[:, None]).T / 6.0).reshape(
        8, 128, OUT).astype(ml_dtypes.bfloat16)
    bp = np.ascontiguousarray(np.broadcast_to(b_p - rm_p * s_p, (128, OUT)))

    biasT = attn_bias[:, bias_idxs].transpose(0, 2, 1)                 # [H, N, NQ]
    eb_d = np.ascontiguousarray(
        np.exp(biasT).reshape(H, NCH, 128, NQ)).astype(ml_dtypes.bfloat16)

    xs = x[:, _SUB_IDX, :]                                             # [B, NQ, C]
    in_maps = []
    for i in range(NCORES):
        sl = slice(i * BPC, (i + 1) * BPC)
        xt = np.ascontiguousarray(x[sl].transpose(0, 2, 1)).reshape(BPC, 2, 128, N).astype(bf)
        xst = np.ascontiguousarray(xs[sl].transpose(0, 2, 1)).reshape(BPC, 2, 128, NQ).astype(bf)
        in_maps.append({
            "xt": xt, "xst": xst,
            "wkt": wkt, "wvt": wvt, "wqt": wqt, "wpt": wpt,
            "bq": bq, "bv": bvd, "bp": bp,
            "eb": eb_d, "ones": np.ones((128, 128), ml_dtypes.bfloat16),
        })
    return in_maps


def _body(tc, a, out_ap):
    import concourse.bass as bass  # noqa: F401
    import concourse.mybir as mybir
    from contextlib import ExitStack

    nc = tc.nc
    f32 = mybir.dt.float32
    f32r = mybir.dt.float32r
    bf16 = mybir.dt.bfloat16
    AF = mybir.ActivationFunctionType
    ALU = mybir.AluOpType

    def r(ap):
        return ap

    with ExitStack() as ctx:
        ctx.enter_context(
            nc.allow_low_precision(reason="o-side bf16 is deliberate; verified vs fp32 reference")
        )
        singles = ctx.enter_context(tc.tile_pool(name="singles", bufs=1))
        # DMA order matters at startup: the first k-proj matmul only needs
        # wk + xt slice 0, so those go first (xt is issued inside the b loop)
        # wk as two separate tiles: tile-granular DMA dependencies mean the
        # first k-proj matmul (head pair 0) waits only on the first ~370ns half
        wk_a = singles.tile([128, 2, 256], bf16)
        wk_b = singles.tile([128, 2, 256], bf16)
        nc.sync.dma_start(wk_a, a["wkt"].rearrange("c p j -> p c j")[:, :, 0:256])

        def wk_sl(cc, pr):
            t = wk_a if pr < 2 else wk_b
            return t[:, cc, (pr % 2) * 128:(pr % 2) * 128 + 128]
        bqs = singles.tile([128, 4], f32)
        ones = singles.tile([128, 128], bf16)
        wq = singles.tile([128, 2, 512], bf16)
        wv = singles.tile([128, 2, 1024], bf16)
        bvs = singles.tile([128, H], f32)
        wp = singles.tile([128, 8, OUT], bf16)
        bps = singles.tile([128, OUT], f32)
        eb = singles.tile([128, H, NCH, NQ], bf16)   # exp(bias), resident all-run

        xt_p = ctx.enter_context(tc.tile_pool(name="xt", bufs=2))
        xst_p = ctx.enter_context(tc.tile_pool(name="xst", bufs=2))
        kt_p = ctx.enter_context(tc.tile_pool(name="kt", bufs=2))
        v_p = ctx.enter_context(tc.tile_pool(name="v", bufs=1))
        qt_p = ctx.enter_context(tc.tile_pool(name="qt", bufs=2))
        e_p = ctx.enter_context(tc.tile_pool(name="e", bufs=6))
        ep_p = ctx.enter_context(tc.tile_pool(name="ep", bufs=4))
        ot_p = ctx.enter_context(tc.tile_pool(name="ot", bufs=2))
        rc_p = ctx.enter_context(tc.tile_pool(name="rc", bufs=3))
        hs_p = ctx.enter_context(tc.tile_pool(name="hs", bufs=3))
        ob_p = ctx.enter_context(tc.tile_pool(name="ob", bufs=3))
        ps_work = ctx.enter_context(tc.tile_pool(name="ps_work", bufs=3, space="PSUM"))
        ps_o = ctx.enter_context(tc.tile_pool(name="ps_o", bufs=1, space="PSUM"))
        ps_sum = ctx.enter_context(tc.tile_pool(name="ps_sum", bufs=1, space="PSUM"))

        _wt_n = [0]

        def work_tile():
            _wt_n[0] += 1
            return ps_work.tile([128, GRP, 512], f32, tag="w", name=f"wt{_wt_n[0]}")

        # batch-0 startup prefetch: DMA dispatch is the startup bottleneck
        # (~650ns serial SP dispatch each), so use few, need-ordered transfers
        xt0 = xt_p.tile([128, 2, N], bf16, tag="xt0", name="xt0")
        for n0, n1 in ((0, 512), (512, 1024)):
            nc.sync.dma_start(
                xt0[:, :, n0:n1], a["xt"][0, :, :, n0:n1].rearrange("c p n -> p c n")
            )
        nc.sync.dma_start(wk_b, a["wkt"].rearrange("c p j -> p c j")[:, :, 256:512])
        nc.sync.dma_start(
            xt0[:, :, 1024:N], a["xt"][0, :, :, 1024:N].rearrange("c p n -> p c n")
        )

        for b in range(BPC):
            xt = xt0 if b == 0 else xt_p.tile([128, 2, N], bf16)
            if b > 0:
                for ns in range(3):
                    n0 = ns * 512
                    nsz = min(512, N - n0)
                    nc.sync.dma_start(
                        xt[:, :, n0:n0 + nsz],
                        a["xt"][b, :, :, n0:n0 + nsz].rearrange("c p n -> p c n"),
                    )
            xst = xst_p.tile([128, 2, NQ], bf16)
            nc.sync.dma_start(xst, a["xst"][b].rearrange("c p n -> p c n"))
            if b == 0:
                nc.sync.dma_start(wv, a["wvt"].rearrange("c p j -> p c j"))
                nc.sync.dma_start(bqs, a["bq"])
                nc.sync.dma_start(wq, a["wqt"].rearrange("c p j -> p c j"))
                nc.sync.dma_start(ones, a["ones"])
                nc.sync.dma_start(bvs, a["bv"])
                for h in range(H):
                    nc.sync.dma_start(
                        eb[:, h, :, :], a["eb"][h].rearrange("c p q -> p c q")
                    )
                nc.sync.dma_start(wp, a["wpt"].rearrange("c p j -> p c j"))
                nc.sync.dma_start(bps, a["bp"])

            kt = kt_p.tile([128, 4, N], bf16)      # [d(2 heads), pair, n]
            vt = v_p.tile([128, NCH, 1024], bf16)  # [n, chunk, v-ch head-major]
            qt = qt_p.tile([128, 4, NQ], bf16)     # [d(2 heads), pair, q]

            # --- kv/q projections ---
            for pr in range(4):                 # kT: head pairs (no BN bias)
                # two 512-slices share one 2-bank psum tile -> one big copy
                ps2 = work_tile()
                for ns in range(2):
                    n0 = ns * 512
                    for cc in range(2):
                        nc.tensor.matmul(
                            ps2[:, ns, :],
                            lhsT=wk_sl(cc, pr),
                            rhs=r(xt[:, cc, n0:n0 + 512]),
                            start=(cc == 0), stop=(cc == 1),
                        )
                ps = work_tile()[:, 0, :]
                for cc in range(2):
                    nc.tensor.matmul(
                        ps[:, :256],
                        lhsT=wk_sl(cc, pr),
                        rhs=r(xt[:, cc, 1024:N]),
                        start=(cc == 0), stop=(cc == 1),
                    )
                if pr != 3:
                    nc.scalar.copy(kt[:, pr, 0:1024], ps2.rearrange("p a b -> p (a b)"))
                    nc.scalar.copy(kt[:, pr, 1024:N], ps[:, :256])
                else:
                    nc.vector.tensor_copy(kt[:, pr, 0:1024], ps2.rearrange("p a b -> p (a b)"))
                    nc.vector.tensor_copy(kt[:, pr, 1024:N], ps[:, :256])
            for cn in range(NCH):               # v: [n-chunk, 8 heads' v]
                ps2 = work_tile()
                for hf in range(2):
                    for cc in range(2):
                        nc.tensor.matmul(
                            ps2[:, hf, :],
                            lhsT=r(xt[:, cc, cn * 128:(cn + 1) * 128]),
                            rhs=r(wv[:, cc, hf * 512:(hf + 1) * 512]),
                            start=(cc == 0), stop=(cc == 1),
                        )
                if cn % 2 == 0:
                    nc.scalar.copy(vt[:, cn, :], ps2.rearrange("p a b -> p (a b)"))
                else:
                    nc.vector.tensor_copy(vt[:, cn, :], ps2.rearrange("p a b -> p (a b)"))
            for pr in range(4):                 # qT with BN bias fused on ACT
                ps = work_tile()[:, 0, :]
                for cc in range(2):
                    nc.tensor.matmul(
                        ps[:, :NQ],
                        lhsT=r(wq[:, cc, pr * 128:(pr + 1) * 128]),
                        rhs=r(xst[:, cc, :]),
                        start=(cc == 0), stop=(cc == 1),
                    )
                nc.scalar.activation(
                    qt[:, pr, :], ps[:, :NQ], AF.Identity, bias=bqs[:, pr:pr + 1],
                )

            # --- attention per head ---
            ot = ot_p.tile([128, H, NQ], bf16)  # [d, head, q]
            if b % 2 == 0:
                # proj lhsT for a batch PAIR: q=640 = 5x128 exact (vs 2.5x128)
                t2 = hs_p.tile([128, H, 2, NQ], bf16, tag="t2", name=f"t2_{b}")
            t = t2[:, :, b % 2, :]
            for h in range(H):
                pr, p0 = h // 2, 64 * (h % 2)
                po = ps_o.tile([128, NQ], f32)
                psm = ps_sum.tile([128, NQ], f32)
                for g in range(NCH // GRP):
                    sg = work_tile()
                    for j in range(GRP):
                        c = GRP * g + j
                        nc.tensor.matmul(
                            sg[:, j, :NQ],
                            lhsT=r(kt[p0:p0 + 64, pr, c * 128:(c + 1) * 128]),
                            rhs=r(qt[p0:p0 + 64, pr, :]),
                            start=True, stop=True,
                        )
                    e = e_p.tile([128, GRP, NQ], bf16)
                    nc.scalar.activation(e, sg[:, :, :NQ], AF.Exp)
                    # relative-position bias: exp(s+b) = exp(s)*exp(b); bf16
                    # SBUF multiply runs in DVE 2x mode (vs f32 PSUM add)
                    nc.vector.tensor_tensor(
                        e, e, eb[:, h, GRP * g:GRP * (g + 1), :], ALU.mult,
                    )
                    # softmax denominator: pre-add the chunk pair on DVE so the
                    # PE streams one ones-matmul per pair; the LAST group goes
                    # direct to the PE so recip isn't gated on the DVE add
                    if 0 < g < NCH // GRP - 1:
                        epair = ep_p.tile([128, NQ], bf16)
                        nc.vector.tensor_tensor(epair, e[:, 0, :], e[:, 1, :], ALU.add)
                        nc.tensor.matmul(
                            psm,
                            lhsT=r(ones),
                            rhs=r(epair),
                            start=False, stop=False,
                        )
                    else:
                        # first and last group go direct to the PE: the first
                        # isn't gated on the DVE pair-add, and the last lets
                        # recip start without waiting for one
                        for j in range(GRP):
                            nc.tensor.matmul(
                                psm,
                                lhsT=r(ones),
                                rhs=r(e[:, j, :]),
                                start=(g == 0 and j == 0),
                                stop=(g == NCH // GRP - 1 and j == GRP - 1),
                            )
                    for j in range(GRP):
                        c = GRP * g + j
                        nc.tensor.matmul(
                            po,
                            lhsT=r(vt[:, c, h * 128:(h + 1) * 128]),
                            rhs=r(e[:, j, :]),
                            start=(c == 0), stop=(c == NCH - 1),
                        )
                rc = rc_p.tile([128, NQ], bf16)
                nc.vector.reciprocal(rc, psm)
                oh = ot[:, h, :]
                # o = po*(1/sums) with po read straight from PSUM (no copy);
                # bv folds out of A-v (softmax rows sum to 1) and is
                # re-applied inside the hardswish below via AP-scalar operands
                nc.vector.tensor_tensor(oh, po, rc, ALU.mult)
                nc.vector.tensor_scalar_add(oh, oh, bvs[:, h:h + 1])
                th = t[:, h, :]
                # hardswish on the (otherwise idle) Pool engine; it feeds only
                # the end-of-pair output projection, so its latency is hidden
                nc.gpsimd.tensor_scalar(th, oh, 3.0, 6.0, ALU.add, ALU.min)
                nc.gpsimd.tensor_scalar(th, th, 0.0, None, ALU.max)
                nc.gpsimd.tensor_tensor(th, th, oh, ALU.mult)

            # --- output projection (per batch pair, q merged to 640) ---
            if b % 2 == 1:
                out_flat = out_ap.rearrange("b q o -> (b q) o")
                for qc in range(5):
                    r0 = (b - 1) * NQ + qc * 128
                    ps = work_tile()[:, 0, :]
                    for dc in range(8):
                        nc.tensor.matmul(
                            ps[:, :OUT],
                            lhsT=t2[:, dc, :, :].rearrange(
                                "p bb q -> p (bb q)")[:, qc * 128:(qc + 1) * 128],
                            rhs=r(wp[:, dc, :]),
                            start=(dc == 0), stop=(dc == 7),
                        )
                    ob = ob_p.tile([128, OUT], bf16)
                    nc.vector.tensor_tensor(ob, ps[:, :OUT], bps, ALU.add)
                    nc.sync.dma_start(out_flat[r0:r0 + 128, :], ob)


def build():
    import concourse.mybir as mybir
    import concourse.tile as tile
    from concourse import bacc

    nc = bacc.Bacc("TRN2", target_bir_lowering=False, debug=False)
    f32, bf16 = mybir.dt.float32, mybir.dt.bfloat16
    a = {}

    def din(name, shape, dt=f32):
        a[name] = nc.dram_tensor(name, shape, dt, kind="ExternalInput").ap()

    din("xt", [BPC, 2, 128, N], bf16)
    din("xst", [BPC, 2, 128, NQ], bf16)
    din("wkt", [2, 128, 512], bf16)
    din("wvt", [2, 128, 1024], bf16)
    din("wqt", [2, 128, 512], bf16)
    din("wpt", [8, 128, OUT], bf16)
    din("bq", [128, 4])
    din("bv", [128, H])
    din("bp", [128, OUT])
    din("eb", [H, NCH, 128, NQ], bf16)
    din("ones", [128, 128], bf16)
    out_ap = nc.dram_tensor("out", [BPC, NQ, OUT], bf16, kind="ExternalOutput").ap()

    with tile.TileContext(nc) as tc:
        _body(tc, a, out_ap)
    nc.compile()
    return nc


_NC_CACHE = None


def _get_nc():
    global _NC_CACHE
    if _NC_CACHE is None:
        _NC_CACHE = build()
    return _NC_CACHE


def kernel(**inputs):
    from concourse.bass_utils import run_bass_kernel_spmd

    in_maps = _prep(inputs)
    nc = _get_nc()
    res = run_bass_kernel_spmd(nc, in_maps, list(range(NCORES)))
    out = np.concatenate([res.results[i]["out"] for i in range(NCORES)], axis=0)
    return np.ascontiguousarray(out, dtype=np.float32)


if __name__ == "__main__":
    rng = np.random.default_rng(0)
    print("smoke: building bass module...")
    nc = build()
    print("built ok:", sum(len(bb.instructions) for bb in nc.m.functions[0].blocks), "instructions")


# revision 106
# speedup vs baseline: 1.0036x; 1.0036x over previous
"""AttentionSubsample Trainium2 kernel.

Full (unsharded) inputs in, full output out. Data-parallel over batch:
32 batches -> 8 NeuronCores x 4 batches each. Weights/biases replicated.

Per-core dataflow (per batch element), all matmuls bf16 (fp32 PSUM accum):
  kv proj   : kT[d,n] per head-pair + v[n,d] (2-bank psum tiles, one big
              psum->sbuf copy each, split ACT/DVE)
  q proj    : qT[d,q] per head-pair (BN bias fused into the ACT psum->sbuf
              copy; attention scale folded into W_q on host)
  per head  : scoresT[n,q] = kT.T @ qT (PE, K=64) in psum groups of 2
              n-chunks; e = exp(scores) (ACT) -> bf16 sbuf;
              e *= exp(bias) (DVE 2x-mode bf16 mult; exp(bias) is
              precomputed on host and resident in SBUF all run);
              oT[d,q] += v.T @ e (PE);
              softmax denominators: middle chunk-pairs are pre-added on the
              DVE (halves the PE ones-matmul cost), while the first/last
              groups go direct to the PE so the accumulation start isn't
              gated on the DVE and recip can start immediately at head end
  normalize : oT = po * recip(sums) read straight from PSUM (one DVE mult,
              no intermediate copy); hswish on the otherwise-idle Pool
              engine (1/6 folded into W_p on host)
  proj      : out[q,384] = h.T @ WpT + bp per batch PAIR (q=640 = 5x128)
Key algebraic facts used:
  - the k-projection BN bias adds a per-query-column constant to scores,
    which softmax is invariant to -> dropped entirely.
  - the v-channel BN bias folds out: softmax rows sum to 1, so
    attn@(v+bv) = attn@v + bv, applied per-partition after normalization.
  - exp(s+b) = exp(s)*exp(b): the relative-position bias becomes a bf16
    SBUF multiply on the DVE (2x mode) instead of an f32 PSUM add.
Startup is DMA-paced: wk is split into two tiles and batch 0's first x
columns arrive in 256-wide pieces paired with 256-wide first matmuls.
fp8 was evaluated and rejected: the attention here is sharp (scores up to
~9), so per-element e/v quantization errors do not average out (measured
1.7e-2..8e-2 rel err vs the 2e-2 gate).
"""

import sys

if "/opt/trn_rl_repo" not in sys.path:
    sys.path.insert(0, "/opt/trn_rl_repo")

import ml_dtypes
import numpy as np

# --- problem constants (hardcoded, must match the grading reference) ---
B, N, C = 32, 1280, 256
H, KD, D = 8, 64, 128          # heads, key dim, value dim per head
NQ = 320                       # subsampled sequence length
OUT = 384
NCORES = 8
BPC = B // NCORES              # batches per core
EPS = 1e-5
NCH = N // 128                 # 10 n-chunks of 128
GRP = 2                        # scores psum group size (n-chunks per group)

_SUB_IDX = np.concatenate([
    (np.arange(32)[::2][:, None] * 32 + np.arange(32)[::2][None, :]).reshape(-1),
    1024 + (np.arange(16)[::2][:, None] * 16 + np.arange(16)[::2][None, :]).reshape(-1),
])  # [320] subsample row gather


def _prep(inputs):
    """Host-side: fold BN into weights, reorder channels, shard over cores."""
    f32 = np.float32
    x = np.asarray(inputs["x"], f32)
    g_kv, b_kv = np.asarray(inputs["g_kv"], f32), np.asarray(inputs["b_kv"], f32)
    rm_kv, rv_kv = np.asarray(inputs["rm_kv"], f32), np.asarray(inputs["rv_kv"], f32)
    g_q, b_q = np.asarray(inputs["g_q"], f32), np.asarray(inputs["b_q"], f32)
    rm_q, rv_q = np.asarray(inputs["rm_q"], f32), np.asarray(inputs["rv_q"], f32)
    g_p, b_p = np.asarray(inputs["g_p"], f32), np.asarray(inputs["b_p"], f32)
    rm_p, rv_p = np.asarray(inputs["rm_p"], f32), np.asarray(inputs["rv_p"], f32)
    W_kv = np.asarray(inputs["W_kv"], f32)
    W_q = np.asarray(inputs["W_q"], f32)
    W_p = np.asarray(inputs["W_p"], f32)
    attn_bias = np.asarray(inputs["attn_bias"], f32)
    bias_idxs = np.asarray(inputs["bias_idxs"])

    s_kv = g_kv / np.sqrt(rv_kv + EPS)
    Wkv_f = W_kv * s_kv[:, None]
    bkv_f = b_kv - rm_kv * s_kv
    kidx = np.concatenate([np.arange(h * 192, h * 192 + KD) for h in range(H)])
    vidx = np.concatenate([np.arange(h * 192 + KD, (h + 1) * 192) for h in range(H)])
    bf = ml_dtypes.bfloat16
    wkt = np.ascontiguousarray(Wkv_f[kidx].T).reshape(2, 128, 512).astype(bf)
    wvt = np.ascontiguousarray(Wkv_f[vidx].T).reshape(2, 128, 1024).astype(bf)
    # k BN bias dropped: adds a per-q-column constant to scores (softmax-invariant)
    bvd = np.ascontiguousarray(bkv_f[vidx].reshape(8, 128).T)          # [128, H]

    scale = KD ** -0.5
    s_q = g_q / np.sqrt(rv_q + EPS)
    wqt = np.ascontiguousarray((W_q * (s_q * scale)[:, None]).T).reshape(2, 128, 512).astype(bf)
    bq = np.ascontiguousarray(((b_q - rm_q * s_q) * scale).reshape(4, 128).T)

    s_p = g_p / np.sqrt(rv_p + EPS)
    wpt = np.ascontiguousarray((W_p * s_p[:, None]).T / 6.0).reshape(
        8, 128, OUT).astype(ml_dtypes.bfloat16)
    bp = np.ascontiguousarray(np.broadcast_to(b_p - rm_p * s_p, (128, OUT)))

    biasT = attn_bias[:, bias_idxs].transpose(0, 2, 1)                 # [H, N, NQ]
    eb_d = np.ascontiguousarray(
        np.exp(biasT).reshape(H, NCH, 128, NQ)).astype(ml_dtypes.bfloat16)

    xs = x[:, _SUB_IDX, :]                                             # [B, NQ, C]
    in_maps = []
    for i in range(NCORES):
        sl = slice(i * BPC, (i + 1) * BPC)
        xt = np.ascontiguousarray(x[sl].transpose(0, 2, 1)).reshape(BPC, 2, 128, N).astype(bf)
        xst = np.ascontiguousarray(xs[sl].transpose(0, 2, 1)).reshape(BPC, 2, 128, NQ).astype(bf)
        in_maps.append({
            "xt": xt, "xst": xst,
            "wkt": wkt, "wvt": wvt, "wqt": wqt, "wpt": wpt,
            "bq": bq, "bv": bvd, "bp": bp,
            "eb": eb_d, "ones": np.ones((128, 128), ml_dtypes.bfloat16),
        })
    return in_maps


def _body(tc, a, out_ap):
    import concourse.bass as bass  # noqa: F401
    import concourse.mybir as mybir
    from contextlib import ExitStack

    nc = tc.nc
    f32 = mybir.dt.float32
    f32r = mybir.dt.float32r
    bf16 = mybir.dt.bfloat16
    AF = mybir.ActivationFunctionType
    ALU = mybir.AluOpType

    def r(ap):
        return ap

    with ExitStack() as ctx:
        ctx.enter_context(
            nc.allow_low_precision(reason="o-side bf16 is deliberate; verified vs fp32 reference")
        )
        singles = ctx.enter_context(tc.tile_pool(name="singles", bufs=1))
        # DMA order matters at startup: the first k-proj matmul only needs
        # wk + xt slice 0, so those go first (xt is issued inside the b loop)
        # wk as two separate tiles: tile-granular DMA dependencies mean the
        # first k-proj matmul (head pair 0) waits only on the first ~370ns half
        wk_a = singles.tile([128, 2, 256], bf16)
        wk_b = singles.tile([128, 2, 256], bf16)
        nc.sync.dma_start(wk_a, a["wkt"].rearrange("c p j -> p c j")[:, :, 0:256])

        def wk_sl(cc, pr):
            t = wk_a if pr < 2 else wk_b
            return t[:, cc, (pr % 2) * 128:(pr % 2) * 128 + 128]
        bqs = singles.tile([128, 4], f32)
        ones = singles.tile([128, 128], bf16)
        wq = singles.tile([128, 2, 512], bf16)
        wv = singles.tile([128, 2, 1024], bf16)
        bvs = singles.tile([128, H], f32)
        wp = singles.tile([128, 8, OUT], bf16)
        bps = singles.tile([128, OUT], f32)
        eb = singles.tile([128, H, NCH, NQ], bf16)   # exp(bias), resident all-run

        xt_p = ctx.enter_context(tc.tile_pool(name="xt", bufs=2))
        xst_p = ctx.enter_context(tc.tile_pool(name="xst", bufs=2))
        kt_p = ctx.enter_context(tc.tile_pool(name="kt", bufs=2))
        v_p = ctx.enter_context(tc.tile_pool(name="v", bufs=1))
        qt_p = ctx.enter_context(tc.tile_pool(name="qt", bufs=2))
        e_p = ctx.enter_context(tc.tile_pool(name="e", bufs=6))
        ep_p = ctx.enter_context(tc.tile_pool(name="ep", bufs=4))
        ot_p = ctx.enter_context(tc.tile_pool(name="ot", bufs=2))
        rc_p = ctx.enter_context(tc.tile_pool(name="rc", bufs=3))
        hs_p = ctx.enter_context(tc.tile_pool(name="hs", bufs=3))
        ob_p = ctx.enter_context(tc.tile_pool(name="ob", bufs=3))
        ps_work = ctx.enter_context(tc.tile_pool(name="ps_work", bufs=3, space="PSUM"))
        ps_o = ctx.enter_context(tc.tile_pool(name="ps_o", bufs=1, space="PSUM"))
        ps_sum = ctx.enter_context(tc.tile_pool(name="ps_sum", bufs=1, space="PSUM"))

        _wt_n = [0]

        def work_tile():
            _wt_n[0] += 1
            return ps_work.tile([128, GRP, 512], f32, tag="w", name=f"wt{_wt_n[0]}")

        # batch-0 startup prefetch: DMA dispatch is the startup bottleneck
        # (~650ns serial SP dispatch each), so use few, need-ordered transfers
        xt0 = xt_p.tile([128, 2, N], bf16, tag="xt0", name="xt0")
        for n0, n1 in ((0, 512), (512, 1024)):
            nc.sync.dma_start(
                xt0[:, :, n0:n1], a["xt"][0, :, :, n0:n1].rearrange("c p n -> p c n")
            )
        nc.sync.dma_start(wk_b, a["wkt"].rearrange("c p j -> p c j")[:, :, 256:512])
        nc.sync.dma_start(
            xt0[:, :, 1024:N], a["xt"][0, :, :, 1024:N].rearrange("c p n -> p c n")
        )

        for b in range(BPC):
            xt = xt0 if b == 0 else xt_p.tile([128, 2, N], bf16)
            if b > 0:
                for ns in range(3):
                    n0 = ns * 512
                    nsz = min(512, N - n0)
                    nc.sync.dma_start(
                        xt[:, :, n0:n0 + nsz],
                        a["xt"][b, :, :, n0:n0 + nsz].rearrange("c p n -> p c n"),
                    )
            xst = xst_p.tile([128, 2, NQ], bf16)
            nc.sync.dma_start(xst, a["xst"][b].rearrange("c p n -> p c n"))
            if b == 0:
                nc.sync.dma_start(wv, a["wvt"].rearrange("c p j -> p c j"))
                nc.sync.dma_start(bqs, a["bq"])
                nc.sync.dma_start(wq, a["wqt"].rearrange("c p j -> p c j"))
                nc.sync.dma_start(ones, a["ones"])
                nc.sync.dma_start(bvs, a["bv"])
                for h in range(H):
                    nc.sync.dma_start(
                        eb[:, h, :, :], a["eb"][h].rearrange("c p q -> p c q")
                    )
                nc.sync.dma_start(wp, a["wpt"].rearrange("c p j -> p c j"))
                nc.sync.dma_start(bps, a["bp"])

            kt = kt_p.tile([128, 4, N], bf16)      # [d(2 heads), pair, n]
            vt = v_p.tile([128, NCH, 1024], bf16)  # [n, chunk, v-ch head-major]
            qt = qt_p.tile([128, 4, NQ], bf16)     # [d(2 heads), pair, q]

            # --- kv/q projections ---
            # emission order k, v0-3, q, v4-9: qt lands before the first
            # head's score matmuls need it, while the late v chunks still
            # beat head 0's attn@v accumulation deadline
            def k_part(pr):
                # two 512-slices share one 2-bank psum tile -> one big copy
                ps2 = work_tile()
                for ns in range(2):
                    n0 = ns * 512
                    for cc in range(2):
                        nc.tensor.matmul(
                            ps2[:, ns, :],
                            lhsT=wk_sl(cc, pr),
                            rhs=r(xt[:, cc, n0:n0 + 512]),
                            start=(cc == 0), stop=(cc == 1),
                        )
                ps = work_tile()[:, 0, :]
                for cc in range(2):
                    nc.tensor.matmul(
                        ps[:, :256],
                        lhsT=wk_sl(cc, pr),
                        rhs=r(xt[:, cc, 1024:N]),
                        start=(cc == 0), stop=(cc == 1),
                    )
                if pr != 3:
                    nc.scalar.copy(kt[:, pr, 0:1024], ps2.rearrange("p a b -> p (a b)"))
                    nc.scalar.copy(kt[:, pr, 1024:N], ps[:, :256])
                else:
                    nc.vector.tensor_copy(kt[:, pr, 0:1024], ps2.rearrange("p a b -> p (a b)"))
                    nc.vector.tensor_copy(kt[:, pr, 1024:N], ps[:, :256])

            def v_part(cn):
                ps2 = work_tile()
                for hf in range(2):
                    for cc in range(2):
                        nc.tensor.matmul(
                            ps2[:, hf, :],
                            lhsT=r(xt[:, cc, cn * 128:(cn + 1) * 128]),
                            rhs=r(wv[:, cc, hf * 512:(hf + 1) * 512]),
                            start=(cc == 0), stop=(cc == 1),
                        )
                if cn % 2 == 0:
                    nc.scalar.copy(vt[:, cn, :], ps2.rearrange("p a b -> p (a b)"))
                else:
                    nc.vector.tensor_copy(vt[:, cn, :], ps2.rearrange("p a b -> p (a b)"))

            def q_part(pr):
                ps = work_tile()[:, 0, :]
                for cc in range(2):
                    nc.tensor.matmul(
                        ps[:, :NQ],
                        lhsT=r(wq[:, cc, pr * 128:(pr + 1) * 128]),
                        rhs=r(xst[:, cc, :]),
                        start=(cc == 0), stop=(cc == 1),
                    )
                nc.scalar.activation(
                    qt[:, pr, :], ps[:, :NQ], AF.Identity, bias=bqs[:, pr:pr + 1],
                )

            for pr in range(4):
                k_part(pr)
            for cn in range(4):
                v_part(cn)
            for pr in range(4):
                q_part(pr)
            for cn in range(4, NCH):
                v_part(cn)

            # --- attention per head ---
            ot = ot_p.tile([128, H, NQ], bf16)  # [d, head, q]
            if b % 2 == 0:
                # proj lhsT for a batch PAIR: q=640 = 5x128 exact (vs 2.5x128)
                t2 = hs_p.tile([128, H, 2, NQ], bf16, tag="t2", name=f"t2_{b}")
            t = t2[:, :, b % 2, :]
            for h in range(H):
                pr, p0 = h // 2, 64 * (h % 2)
                po = ps_o.tile([128, NQ], f32)
                psm = ps_sum.tile([128, NQ], f32)
                for g in range(NCH // GRP):
                    sg = work_tile()
                    for j in range(GRP):
                        c = GRP * g + j
                        nc.tensor.matmul(
                            sg[:, j, :NQ],
                            lhsT=r(kt[p0:p0 + 64, pr, c * 128:(c + 1) * 128]),
                            rhs=r(qt[p0:p0 + 64, pr, :]),
                            start=True, stop=True,
                        )
                    e = e_p.tile([128, GRP, NQ], bf16)
                    nc.scalar.activation(e, sg[:, :, :NQ], AF.Exp)
                    # relative-position bias: exp(s+b) = exp(s)*exp(b); bf16
                    # SBUF multiply runs in DVE 2x mode (vs f32 PSUM add)
                    nc.vector.tensor_tensor(
                        e, e, eb[:, h, GRP * g:GRP * (g + 1), :], ALU.mult,
                    )
                    # softmax denominator: pre-add the chunk pair on DVE so the
                    # PE streams one ones-matmul per pair; the LAST group goes
                    # direct to the PE so recip isn't gated on the DVE add
                    if 0 < g < NCH // GRP - 1:
                        epair = ep_p.tile([128, NQ], bf16)
                        nc.vector.tensor_tensor(epair, e[:, 0, :], e[:, 1, :], ALU.add)
                        nc.tensor.matmul(
                            psm,
                            lhsT=r(ones),
                            rhs=r(epair),
                            start=False, stop=False,
                        )
                    else:
                        # first and last group go direct to the PE: the first
                        # isn't gated on the DVE pair-add, and the last lets
                        # recip start without waiting for one
                        for j in range(GRP):
                            nc.tensor.matmul(
                                psm,
                                lhsT=r(ones),
                                rhs=r(e[:, j, :]),
                                start=(g == 0 and j == 0),
                                stop=(g == NCH // GRP - 1 and j == GRP - 1),
                            )
                    for j in range(GRP):
                        c = GRP * g + j
                        nc.tensor.matmul(
                            po,
                            lhsT=r(vt[:, c, h * 128:(h + 1) * 128]),
                            rhs=r(e[:, j, :]),
                            start=(c == 0), stop=(c == NCH - 1),
                        )
                rc = rc_p.tile([128, NQ], bf16)
                nc.vector.reciprocal(rc, psm)
                oh = ot[:, h, :]
                # o = po*(1/sums) with po read straight from PSUM (no copy);
                # bv folds out of A-v (softmax rows sum to 1) and is
                # re-applied inside the hardswish below via AP-scalar operands
                nc.vector.tensor_tensor(oh, po, rc, ALU.mult)
                nc.vector.tensor_scalar_add(oh, oh, bvs[:, h:h + 1])
                th = t[:, h, :]
                # hardswish on the (otherwise idle) Pool engine; it feeds only
                # the end-of-pair output projection, so its latency is hidden
                nc.gpsimd.tensor_scalar(th, oh, 3.0, 6.0, ALU.add, ALU.min)
                nc.gpsimd.tensor_scalar(th, th, 0.0, None, ALU.max)
                nc.gpsimd.tensor_tensor(th, th, oh, ALU.mult)

            # --- output projection (per batch pair, q merged to 640) ---
            if b % 2 == 1:
                out_flat = out_ap.rearrange("b q o -> (b q) o")
                for qc in range(5):
                    r0 = (b - 1) * NQ + qc * 128
                    ps = work_tile()[:, 0, :]
                    for dc in range(8):
                        nc.tensor.matmul(
                            ps[:, :OUT],
                            lhsT=t2[:, dc, :, :].rearrange(
                                "p bb q -> p (bb q)")[:, qc * 128:(qc + 1) * 128],
                            rhs=r(wp[:, dc, :]),
                            start=(dc == 0), stop=(dc == 7),
                        )
                    ob = ob_p.tile([128, OUT], bf16)
                    nc.vector.tensor_tensor(ob, ps[:, :OUT], bps, ALU.add)
                    nc.sync.dma_start(out_flat[r0:r0 + 128, :], ob)


def build():
    import concourse.mybir as mybir
    import concourse.tile as tile
    from concourse import bacc

    nc = bacc.Bacc("TRN2", target_bir_lowering=False, debug=False)
    f32, bf16 = mybir.dt.float32, mybir.dt.bfloat16
    a = {}

    def din(name, shape, dt=f32):
        a[name] = nc.dram_tensor(name, shape, dt, kind="ExternalInput").ap()

    din("xt", [BPC, 2, 128, N], bf16)
    din("xst", [BPC, 2, 128, NQ], bf16)
    din("wkt", [2, 128, 512], bf16)
    din("wvt", [2, 128, 1024], bf16)
    din("wqt", [2, 128, 512], bf16)
    din("wpt", [8, 128, OUT], bf16)
    din("bq", [128, 4])
    din("bv", [128, H])
    din("bp", [128, OUT])
    din("eb", [H, NCH, 128, NQ], bf16)
    din("ones", [128, 128], bf16)
    out_ap = nc.dram_tensor("out", [BPC, NQ, OUT], bf16, kind="ExternalOutput").ap()

    with tile.TileContext(nc) as tc:
        _body(tc, a, out_ap)
    nc.compile()
    return nc


_NC_CACHE = None


def _get_nc():
    global _NC_CACHE
    if _NC_CACHE is None:
        _NC_CACHE = build()
    return _NC_CACHE


def kernel(**inputs):
    from concourse.bass_utils import run_bass_kernel_spmd

    in_maps = _prep(inputs)
    nc = _get_nc()
    res = run_bass_kernel_spmd(nc, in_maps, list(range(NCORES)))
    out = np.concatenate([res.results[i]["out"] for i in range(NCORES)], axis=0)
    return np.ascontiguousarray(out, dtype=np.float32)


if __name__ == "__main__":
    rng = np.random.default_rng(0)
    print("smoke: building bass module...")
    nc = build()
    print("built ok:", sum(len(bb.instructions) for bb in nc.m.functions[0].blocks), "instructions")


# revision 113
# speedup vs baseline: 1.0123x; 1.0088x over previous
"""AttentionSubsample Trainium2 kernel.

Full (unsharded) inputs in, full output out. Data-parallel over batch:
32 batches -> 8 NeuronCores x 4 batches each. Weights/biases replicated.

Per-core dataflow (per batch element), all matmuls bf16 (fp32 PSUM accum):
  kv proj   : kT[d,n] per head-pair + v[n,d] (2-bank psum tiles, one big
              psum->sbuf copy each, split ACT/DVE)
  q proj    : qT[d,q] per head-pair (BN bias fused into the ACT psum->sbuf
              copy; attention scale folded into W_q on host)
  per head  : scoresT[n,q] = kT.T @ qT (PE, K=64) in psum groups of 2
              n-chunks; e = exp(scores) (ACT) -> bf16 sbuf;
              e *= exp(bias) (DVE 2x-mode bf16 mult; exp(bias) is
              precomputed on host and resident in SBUF all run);
              oT[d,q] += v.T @ e (PE);
              softmax denominators: middle chunk-pairs are pre-added on the
              DVE (halves the PE ones-matmul cost), while the first/last
              groups go direct to the PE so the accumulation start isn't
              gated on the DVE and recip can start immediately at head end
  normalize : oT = po * recip(sums) read straight from PSUM (one DVE mult,
              no intermediate copy); hswish on the otherwise-idle Pool
              engine (1/6 folded into W_p on host)
  proj      : out[q,384] = h.T @ WpT + bp per batch PAIR (q=640 = 5x128)
Key algebraic facts used:
  - the k-projection BN bias adds a per-query-column constant to scores,
    which softmax is invariant to -> dropped entirely.
  - the v-channel BN bias folds out: softmax rows sum to 1, so
    attn@(v+bv) = attn@v + bv, applied per-partition after normalization.
  - exp(s+b) = exp(s)*exp(b): the relative-position bias becomes a bf16
    SBUF multiply on the DVE (2x mode) instead of an f32 PSUM add.
Startup is DMA-paced: wk is split into two tiles and batch 0's first x
columns arrive in 256-wide pieces paired with 256-wide first matmuls.
fp8 was evaluated and rejected: the attention here is sharp (scores up to
~9), so per-element e/v quantization errors do not average out (measured
1.7e-2..8e-2 rel err vs the 2e-2 gate).
"""

import sys

if "/opt/trn_rl_repo" not in sys.path:
    sys.path.insert(0, "/opt/trn_rl_repo")

import ml_dtypes
import numpy as np

# --- problem constants (hardcoded, must match the grading reference) ---
B, N, C = 32, 1280, 256
H, KD, D = 8, 64, 128          # heads, key dim, value dim per head
NQ = 320                       # subsampled sequence length
OUT = 384
NCORES = 8
BPC = B // NCORES              # batches per core
EPS = 1e-5
NCH = N // 128                 # 10 n-chunks of 128
GRP = 2                        # scores psum group size (n-chunks per group)

_SUB_IDX = np.concatenate([
    (np.arange(32)[::2][:, None] * 32 + np.arange(32)[::2][None, :]).reshape(-1),
    1024 + (np.arange(16)[::2][:, None] * 16 + np.arange(16)[::2][None, :]).reshape(-1),
])  # [320] subsample row gather


def _prep(inputs):
    """Host-side: fold BN into weights, reorder channels, shard over cores."""
    f32 = np.float32
    x = np.asarray(inputs["x"], f32)
    g_kv, b_kv = np.asarray(inputs["g_kv"], f32), np.asarray(inputs["b_kv"], f32)
    rm_kv, rv_kv = np.asarray(inputs["rm_kv"], f32), np.asarray(inputs["rv_kv"], f32)
    g_q, b_q = np.asarray(inputs["g_q"], f32), np.asarray(inputs["b_q"], f32)
    rm_q, rv_q = np.asarray(inputs["rm_q"], f32), np.asarray(inputs["rv_q"], f32)
    g_p, b_p = np.asarray(inputs["g_p"], f32), np.asarray(inputs["b_p"], f32)
    rm_p, rv_p = np.asarray(inputs["rm_p"], f32), np.asarray(inputs["rv_p"], f32)
    W_kv = np.asarray(inputs["W_kv"], f32)
    W_q = np.asarray(inputs["W_q"], f32)
    W_p = np.asarray(inputs["W_p"], f32)
    attn_bias = np.asarray(inputs["attn_bias"], f32)
    bias_idxs = np.asarray(inputs["bias_idxs"])

    s_kv = g_kv / np.sqrt(rv_kv + EPS)
    Wkv_f = W_kv * s_kv[:, None]
    bkv_f = b_kv - rm_kv * s_kv
    kidx = np.concatenate([np.arange(h * 192, h * 192 + KD) for h in range(H)])
    vidx = np.concatenate([np.arange(h * 192 + KD, (h + 1) * 192) for h in range(H)])
    bf = ml_dtypes.bfloat16
    wkt = np.ascontiguousarray(Wkv_f[kidx].T).reshape(2, 128, 512).astype(bf)
    wvt = np.ascontiguousarray(Wkv_f[vidx].T).reshape(2, 128, 1024).astype(bf)
    # k BN bias dropped: adds a per-q-column constant to scores (softmax-invariant)
    bvd = np.ascontiguousarray(bkv_f[vidx].reshape(8, 128).T)          # [128, H]

    scale = KD ** -0.5
    s_q = g_q / np.sqrt(rv_q + EPS)
    wqt = np.ascontiguousarray((W_q * (s_q * scale)[:, None]).T).reshape(2, 128, 512).astype(bf)
    bq = np.ascontiguousarray(((b_q - rm_q * s_q) * scale).reshape(4, 128).T)

    s_p = g_p / np.sqrt(rv_p + EPS)
    wpt = np.ascontiguousarray((W_p * s_p[:, None]).T / 6.0).reshape(
        8, 128, OUT).astype(ml_dtypes.bfloat16)
    bp = np.ascontiguousarray(np.broadcast_to(b_p - rm_p * s_p, (128, OUT)))

    biasT = attn_bias[:, bias_idxs].transpose(0, 2, 1)                 # [H, N, NQ]
    eb_d = np.ascontiguousarray(
        np.exp(biasT).reshape(H, NCH, 128, NQ)).astype(ml_dtypes.bfloat16)

    xs = x[:, _SUB_IDX, :]                                             # [B, NQ, C]
    in_maps = []
    for i in range(NCORES):
        sl = slice(i * BPC, (i + 1) * BPC)
        xt = np.ascontiguousarray(x[sl].transpose(0, 2, 1)).reshape(BPC, 2, 128, N).astype(bf)
        xst = np.ascontiguousarray(xs[sl].transpose(0, 2, 1)).reshape(BPC, 2, 128, NQ).astype(bf)
        in_maps.append({
            "xt": xt, "xst": xst,
            "wkt": wkt, "wvt": wvt, "wqt": wqt, "wpt": wpt,
            "bq": bq, "bv": bvd, "bp": bp,
            "eb": eb_d, "ones": np.ones((128, 128), ml_dtypes.bfloat16),
        })
    return in_maps


def _body(tc, a, out_ap):
    import concourse.bass as bass  # noqa: F401
    import concourse.mybir as mybir
    from contextlib import ExitStack

    nc = tc.nc
    f32 = mybir.dt.float32
    f32r = mybir.dt.float32r
    bf16 = mybir.dt.bfloat16
    AF = mybir.ActivationFunctionType
    ALU = mybir.AluOpType

    def r(ap):
        return ap

    with ExitStack() as ctx:
        ctx.enter_context(
            nc.allow_low_precision(reason="o-side bf16 is deliberate; verified vs fp32 reference")
        )
        singles = ctx.enter_context(tc.tile_pool(name="singles", bufs=1))
        # DMA order matters at startup: the first k-proj matmul only needs
        # wk + xt slice 0, so those go first (xt is issued inside the b loop)
        # wk as two separate tiles: tile-granular DMA dependencies mean the
        # first k-proj matmul (head pair 0) waits only on the first ~370ns half
        wk_a = singles.tile([128, 2, 256], bf16)
        wk_b = singles.tile([128, 2, 256], bf16)
        nc.sync.dma_start(wk_a, a["wkt"].rearrange("c p j -> p c j")[:, :, 0:256])

        def wk_sl(cc, pr):
            t = wk_a if pr < 2 else wk_b
            return t[:, cc, (pr % 2) * 128:(pr % 2) * 128 + 128]
        bqs = singles.tile([128, 4], f32)
        ones = singles.tile([128, 128], bf16)
        wq = singles.tile([128, 2, 512], bf16)
        wv = singles.tile([128, 2, 1024], bf16)
        bvs = singles.tile([128, H], f32)
        wp = singles.tile([128, 8, OUT], bf16)
        bps = singles.tile([128, OUT], f32)
        eb = singles.tile([128, H, NCH, NQ], bf16)   # exp(bias), resident all-run

        xt_p = ctx.enter_context(tc.tile_pool(name="xt", bufs=2))
        xst_p = ctx.enter_context(tc.tile_pool(name="xst", bufs=2))
        kt_p = ctx.enter_context(tc.tile_pool(name="kt", bufs=2))
        v_p = ctx.enter_context(tc.tile_pool(name="v", bufs=1))
        qt_p = ctx.enter_context(tc.tile_pool(name="qt", bufs=2))
        e_p = ctx.enter_context(tc.tile_pool(name="e", bufs=6))
        ep_p = ctx.enter_context(tc.tile_pool(name="ep", bufs=4))
        ot_p = ctx.enter_context(tc.tile_pool(name="ot", bufs=2))
        rc_p = ctx.enter_context(tc.tile_pool(name="rc", bufs=3))
        hs_p = ctx.enter_context(tc.tile_pool(name="hs", bufs=3))
        ob_p = ctx.enter_context(tc.tile_pool(name="ob", bufs=3))
        ps_work = ctx.enter_context(tc.tile_pool(name="ps_work", bufs=3, space="PSUM"))
        ps_o = ctx.enter_context(tc.tile_pool(name="ps_o", bufs=1, space="PSUM"))
        ps_sum = ctx.enter_context(tc.tile_pool(name="ps_sum", bufs=1, space="PSUM"))

        _wt_n = [0]

        def work_tile():
            _wt_n[0] += 1
            return ps_work.tile([128, GRP, 512], f32, tag="w", name=f"wt{_wt_n[0]}")

        # batch-0 startup prefetch: DMA dispatch is the startup bottleneck
        # (~650ns serial SP dispatch each), so use few, need-ordered transfers
        xt0 = xt_p.tile([128, 2, N], bf16, tag="xt0", name="xt0")
        for n0, n1 in ((0, 512), (512, 1024)):
            nc.sync.dma_start(
                xt0[:, :, n0:n1], a["xt"][0, :, :, n0:n1].rearrange("c p n -> p c n")
            )
        nc.sync.dma_start(wk_b, a["wkt"].rearrange("c p j -> p c j")[:, :, 256:512])
        nc.sync.dma_start(
            xt0[:, :, 1024:N], a["xt"][0, :, :, 1024:N].rearrange("c p n -> p c n")
        )

        for b in range(BPC):
            xt = xt0 if b == 0 else xt_p.tile([128, 2, N], bf16)
            if b > 0:
                for ns in range(3):
                    n0 = ns * 512
                    nsz = min(512, N - n0)
                    nc.sync.dma_start(
                        xt[:, :, n0:n0 + nsz],
                        a["xt"][b, :, :, n0:n0 + nsz].rearrange("c p n -> p c n"),
                    )
            xst = xst_p.tile([128, 2, NQ], bf16)
            nc.sync.dma_start(xst, a["xst"][b].rearrange("c p n -> p c n"))
            if b == 0:
                nc.sync.dma_start(wv, a["wvt"].rearrange("c p j -> p c j"))
                nc.sync.dma_start(bqs, a["bq"])
                nc.sync.dma_start(wq, a["wqt"].rearrange("c p j -> p c j"))
                nc.sync.dma_start(ones, a["ones"])
                nc.sync.dma_start(bvs, a["bv"])
                for h in range(H):
                    nc.sync.dma_start(
                        eb[:, h, :, :], a["eb"][h].rearrange("c p q -> p c q")
                    )
                nc.sync.dma_start(wp, a["wpt"].rearrange("c p j -> p c j"))
                nc.sync.dma_start(bps, a["bp"])

            kt = kt_p.tile([128, 4, N], bf16)      # [d(2 heads), pair, n]
            vt = v_p.tile([128, NCH, 1024], bf16)  # [n, chunk, v-ch head-major]
            qt = qt_p.tile([128, 4, NQ], bf16)     # [d(2 heads), pair, q]

            # --- kv/q projections ---
            # emission order k, v0-3, q, v4-9: qt lands before the first
            # head's score matmuls need it, while the late v chunks still
            # beat head 0's attn@v accumulation deadline
            def k_part(pr):
                # two 512-slices share one 2-bank psum tile -> one big copy
                ps2 = work_tile()
                for ns in range(2):
                    n0 = ns * 512
                    for cc in range(2):
                        nc.tensor.matmul(
                            ps2[:, ns, :],
                            lhsT=wk_sl(cc, pr),
                            rhs=r(xt[:, cc, n0:n0 + 512]),
                            start=(cc == 0), stop=(cc == 1),
                        )
                ps = work_tile()[:, 0, :]
                for cc in range(2):
                    nc.tensor.matmul(
                        ps[:, :256],
                        lhsT=wk_sl(cc, pr),
                        rhs=r(xt[:, cc, 1024:N]),
                        start=(cc == 0), stop=(cc == 1),
                    )
                if pr != 3:
                    nc.scalar.copy(kt[:, pr, 0:1024], ps2.rearrange("p a b -> p (a b)"))
                    nc.scalar.copy(kt[:, pr, 1024:N], ps[:, :256])
                else:
                    nc.vector.tensor_copy(kt[:, pr, 0:1024], ps2.rearrange("p a b -> p (a b)"))
                    nc.vector.tensor_copy(kt[:, pr, 1024:N], ps[:, :256])

            def v_part(cn):
                ps2 = work_tile()
                for hf in range(2):
                    for cc in range(2):
                        nc.tensor.matmul(
                            ps2[:, hf, :],
                            lhsT=r(xt[:, cc, cn * 128:(cn + 1) * 128]),
                            rhs=r(wv[:, cc, hf * 512:(hf + 1) * 512]),
                            start=(cc == 0), stop=(cc == 1),
                        )
                if cn % 2 == 0:
                    nc.scalar.copy(vt[:, cn, :], ps2.rearrange("p a b -> p (a b)"))
                else:
                    nc.vector.tensor_copy(vt[:, cn, :], ps2.rearrange("p a b -> p (a b)"))

            def q_part(pr):
                ps = work_tile()[:, 0, :]
                for cc in range(2):
                    nc.tensor.matmul(
                        ps[:, :NQ],
                        lhsT=r(wq[:, cc, pr * 128:(pr + 1) * 128]),
                        rhs=r(xst[:, cc, :]),
                        start=(cc == 0), stop=(cc == 1),
                    )
                nc.scalar.activation(
                    qt[:, pr, :], ps[:, :NQ], AF.Identity, bias=bqs[:, pr:pr + 1],
                )

            for pr in range(4):
                k_part(pr)
            for cn in range(6):
                v_part(cn)
            for pr in range(4):
                q_part(pr)
            for cn in range(6, NCH):
                v_part(cn)

            # --- attention per head ---
            ot = ot_p.tile([128, H, NQ], bf16)  # [d, head, q]
            if b % 2 == 0:
                # proj lhsT for a batch PAIR: q=640 = 5x128 exact (vs 2.5x128)
                t2 = hs_p.tile([128, H, 2, NQ], bf16, tag="t2", name=f"t2_{b}")
            t = t2[:, :, b % 2, :]
            for h in range(H):
                pr, p0 = h // 2, 64 * (h % 2)
                po = ps_o.tile([128, NQ], f32)
                psm = ps_sum.tile([128, NQ], f32)
                for g in range(NCH // GRP):
                    sg = work_tile()
                    for j in range(GRP):
                        c = GRP * g + j
                        nc.tensor.matmul(
                            sg[:, j, :NQ],
                            lhsT=r(kt[p0:p0 + 64, pr, c * 128:(c + 1) * 128]),
                            rhs=r(qt[p0:p0 + 64, pr, :]),
                            start=True, stop=True,
                        )
                    e = e_p.tile([128, GRP, NQ], bf16)
                    nc.scalar.activation(e, sg[:, :, :NQ], AF.Exp)
                    # relative-position bias: exp(s+b) = exp(s)*exp(b); bf16
                    # SBUF multiply runs in DVE 2x mode (vs f32 PSUM add)
                    nc.vector.tensor_tensor(
                        e, e, eb[:, h, GRP * g:GRP * (g + 1), :], ALU.mult,
                    )
                    # softmax denominator: pre-add the chunk pair on DVE so the
                    # PE streams one ones-matmul per pair; the LAST group goes
                    # direct to the PE so recip isn't gated on the DVE add
                    if 0 < g < NCH // GRP - 1:
                        epair = ep_p.tile([128, NQ], bf16)
                        nc.vector.tensor_tensor(epair, e[:, 0, :], e[:, 1, :], ALU.add)
                        nc.tensor.matmul(
                            psm,
                            lhsT=r(ones),
                            rhs=r(epair),
                            start=False, stop=False,
                        )
                    else:
                        # first and last group go direct to the PE: the first
                        # isn't gated on the DVE pair-add, and the last lets
                        # recip start without waiting for one
                        for j in range(GRP):
                            nc.tensor.matmul(
                                psm,
                                lhsT=r(ones),
                                rhs=r(e[:, j, :]),
                                start=(g == 0 and j == 0),
                                stop=(g == NCH // GRP - 1 and j == GRP - 1),
                            )
                    for j in range(GRP):
                        c = GRP * g + j
                        nc.tensor.matmul(
                            po,
                            lhsT=r(vt[:, c, h * 128:(h + 1) * 128]),
                            rhs=r(e[:, j, :]),
                            start=(c == 0), stop=(c == NCH - 1),
                        )
                rc = rc_p.tile([128, NQ], bf16)
                nc.vector.reciprocal(rc, psm)
                oh = ot[:, h, :]
                # o = po*(1/sums) with po read straight from PSUM (no copy);
                # bv folds out of A-v (softmax rows sum to 1) and is
                # re-applied inside the hardswish below via AP-scalar operands
                nc.vector.tensor_tensor(oh, po, rc, ALU.mult)
                nc.vector.tensor_scalar_add(oh, oh, bvs[:, h:h + 1])
                th = t[:, h, :]
                # hardswish on the (otherwise idle) Pool engine; its latency
                # is hidden EXCEPT for the pair's last heads, whose t2 slices
                # the output projection needs right away -> those go on the
                # (by then idle) DVE's much faster queue
                eng = nc.vector if (b % 2 == 1 and h >= 6) else nc.gpsimd
                eng.tensor_scalar(th, oh, 3.0, 6.0, ALU.add, ALU.min)
                eng.tensor_scalar(th, th, 0.0, None, ALU.max)
                eng.tensor_tensor(th, th, oh, ALU.mult)

            # --- output projection (per batch pair, q merged to 640) ---
            if b % 2 == 1:
                out_flat = out_ap.rearrange("b q o -> (b q) o")
                for qc in range(5):
                    r0 = (b - 1) * NQ + qc * 128
                    ps = work_tile()[:, 0, :]
                    for dc in range(8):
                        nc.tensor.matmul(
                            ps[:, :OUT],
                            lhsT=t2[:, dc, :, :].rearrange(
                                "p bb q -> p (bb q)")[:, qc * 128:(qc + 1) * 128],
                            rhs=r(wp[:, dc, :]),
                            start=(dc == 0), stop=(dc == 7),
                        )
                    ob = ob_p.tile([128, OUT], bf16)
                    nc.vector.tensor_tensor(ob, ps[:, :OUT], bps, ALU.add)
                    nc.sync.dma_start(out_flat[r0:r0 + 128, :], ob)


def build():
    import concourse.mybir as mybir
    import concourse.tile as tile
    from concourse import bacc

    nc = bacc.Bacc("TRN2", target_bir_lowering=False, debug=False)
    f32, bf16 = mybir.dt.float32, mybir.dt.bfloat16
    a = {}

    def din(name, shape, dt=f32):
        a[name] = nc.dram_tensor(name, shape, dt, kind="ExternalInput").ap()

    din("xt", [BPC, 2, 128, N], bf16)
    din("xst", [BPC, 2, 128, NQ], bf16)
    din("wkt", [2, 128, 512], bf16)
    din("wvt", [2, 128, 1024], bf16)
    din("wqt", [2, 128, 512], bf16)
    din("wpt", [8, 128, OUT], bf16)
    din("bq", [128, 4])
    din("bv", [128, H])
    din("bp", [128, OUT])
    din("eb", [H, NCH, 128, NQ], bf16)
    din("ones", [128, 128], bf16)
    out_ap = nc.dram_tensor("out", [BPC, NQ, OUT], bf16, kind="ExternalOutput").ap()

    with tile.TileContext(nc) as tc:
        _body(tc, a, out_ap)
    nc.compile()
    return nc


_NC_CACHE = None


def _get_nc():
    global _NC_CACHE
    if _NC_CACHE is None:
        _NC_CACHE = build()
    return _NC_CACHE


def kernel(**inputs):
    from concourse.bass_utils import run_bass_kernel_spmd

    in_maps = _prep(inputs)
    nc = _get_nc()
    res = run_bass_kernel_spmd(nc, in_maps, list(range(NCORES)))
    out = np.concatenate([res.results[i]["out"] for i in range(NCORES)], axis=0)
    return np.ascontiguousarray(out, dtype=np.float32)


if __name__ == "__main__":
    rng = np.random.default_rng(0)
    print("smoke: building bass module...")
    nc = build()
    print("built ok:", sum(len(bb.instructions) for bb in nc.m.functions[0].blocks), "instructions")


# revision 120
# speedup vs baseline: 1.0188x; 1.0064x over previous
"""AttentionSubsample Trainium2 kernel.

Full (unsharded) inputs in, full output out. Data-parallel over batch:
32 batches -> 8 NeuronCores x 4 batches each. Weights/biases replicated.

Per-core dataflow (per batch element), all matmuls bf16 (fp32 PSUM accum):
  kv proj   : kT[d,n] per head-pair + v[n,d] (2-bank psum tiles, one big
              psum->sbuf copy each, split ACT/DVE)
  q proj    : qT[d,q] per head-pair (BN bias fused into the ACT psum->sbuf
              copy; attention scale folded into W_q on host)
  per head  : scoresT[n,q] = kT.T @ qT (PE, K=64) in psum groups of 2
              n-chunks; e = exp(scores) (ACT) -> bf16 sbuf;
              e *= exp(bias) (DVE 2x-mode bf16 mult; exp(bias) is
              precomputed on host and resident in SBUF all run);
              oT[d,q] += v.T @ e (PE);
              softmax denominators: middle chunk-pairs are pre-added on the
              DVE (halves the PE ones-matmul cost), while the first/last
              groups go direct to the PE so the accumulation start isn't
              gated on the DVE and recip can start immediately at head end
  normalize : oT = po * recip(sums) read straight from PSUM (one DVE mult,
              no intermediate copy); hswish on the otherwise-idle Pool
              engine (1/6 folded into W_p on host)
  proj      : out[q,384] = h.T @ WpT + bp per batch PAIR (q=640 = 5x128)
Key algebraic facts used:
  - the k-projection BN bias adds a per-query-column constant to scores,
    which softmax is invariant to -> dropped entirely.
  - the v-channel BN bias folds out: softmax rows sum to 1, so
    attn@(v+bv) = attn@v + bv, applied per-partition after normalization.
  - exp(s+b) = exp(s)*exp(b): the relative-position bias becomes a bf16
    SBUF multiply on the DVE (2x mode) instead of an f32 PSUM add.
Scheduling notes: startup DMAs are few/large/need-ordered (SP dispatch is
650ns serial per transfer); projections emit as k, v0-5, q, v6-9 so qt
lands just before the first head's scores need it; the pair's last two
heads' hardswish runs on the DVE (idle by then) so the output projection
isn't stalled by the Pool queue; everything is double-buffered except vt
(whole-batch reuse) and the 3-buffer PSUM work pool (8-bank limit).
fp8 was evaluated and rejected: the attention here is sharp (scores up to
~9), so per-element e/v quantization errors do not average out (measured
1.7e-2..8e-2 rel err vs the 2e-2 gate).
"""

import sys

if "/opt/trn_rl_repo" not in sys.path:
    sys.path.insert(0, "/opt/trn_rl_repo")

import ml_dtypes
import numpy as np

# --- problem constants (hardcoded, must match the grading reference) ---
B, N, C = 32, 1280, 256
H, KD, D = 8, 64, 128          # heads, key dim, value dim per head
NQ = 320                       # subsampled sequence length
OUT = 384
NCORES = 8
BPC = B // NCORES              # batches per core
EPS = 1e-5
NCH = N // 128                 # 10 n-chunks of 128
GRP = 2                        # scores psum group size (n-chunks per group)

_SUB_IDX = np.concatenate([
    (np.arange(32)[::2][:, None] * 32 + np.arange(32)[::2][None, :]).reshape(-1),
    1024 + (np.arange(16)[::2][:, None] * 16 + np.arange(16)[::2][None, :]).reshape(-1),
])  # [320] subsample row gather


def _prep(inputs):
    """Host-side: fold BN into weights, reorder channels, shard over cores."""
    f32 = np.float32
    x = np.asarray(inputs["x"], f32)
    g_kv, b_kv = np.asarray(inputs["g_kv"], f32), np.asarray(inputs["b_kv"], f32)
    rm_kv, rv_kv = np.asarray(inputs["rm_kv"], f32), np.asarray(inputs["rv_kv"], f32)
    g_q, b_q = np.asarray(inputs["g_q"], f32), np.asarray(inputs["b_q"], f32)
    rm_q, rv_q = np.asarray(inputs["rm_q"], f32), np.asarray(inputs["rv_q"], f32)
    g_p, b_p = np.asarray(inputs["g_p"], f32), np.asarray(inputs["b_p"], f32)
    rm_p, rv_p = np.asarray(inputs["rm_p"], f32), np.asarray(inputs["rv_p"], f32)
    W_kv = np.asarray(inputs["W_kv"], f32)
    W_q = np.asarray(inputs["W_q"], f32)
    W_p = np.asarray(inputs["W_p"], f32)
    attn_bias = np.asarray(inputs["attn_bias"], f32)
    bias_idxs = np.asarray(inputs["bias_idxs"])

    s_kv = g_kv / np.sqrt(rv_kv + EPS)
    Wkv_f = W_kv * s_kv[:, None]
    bkv_f = b_kv - rm_kv * s_kv
    kidx = np.concatenate([np.arange(h * 192, h * 192 + KD) for h in range(H)])
    vidx = np.concatenate([np.arange(h * 192 + KD, (h + 1) * 192) for h in range(H)])
    bf = ml_dtypes.bfloat16
    wkt = np.ascontiguousarray(Wkv_f[kidx].T).reshape(2, 128, 512).astype(bf)
    wvt = np.ascontiguousarray(Wkv_f[vidx].T).reshape(2, 128, 1024).astype(bf)
    # k BN bias dropped: adds a per-q-column constant to scores (softmax-invariant)
    bvd = np.ascontiguousarray(bkv_f[vidx].reshape(8, 128).T)          # [128, H]

    scale = KD ** -0.5
    s_q = g_q / np.sqrt(rv_q + EPS)
    wqt = np.ascontiguousarray((W_q * (s_q * scale)[:, None]).T).reshape(2, 128, 512).astype(bf)
    bq = np.ascontiguousarray(((b_q - rm_q * s_q) * scale).reshape(4, 128).T)

    s_p = g_p / np.sqrt(rv_p + EPS)
    wpt = np.ascontiguousarray((W_p * s_p[:, None]).T / 6.0).reshape(
        8, 128, OUT).astype(ml_dtypes.bfloat16)
    bp = np.ascontiguousarray(np.broadcast_to(b_p - rm_p * s_p, (128, OUT)))

    biasT = attn_bias[:, bias_idxs].transpose(0, 2, 1)                 # [H, N, NQ]
    eb_d = np.ascontiguousarray(
        np.exp(biasT).reshape(H, NCH, 128, NQ)).astype(ml_dtypes.bfloat16)

    xs = x[:, _SUB_IDX, :]                                             # [B, NQ, C]
    in_maps = []
    for i in range(NCORES):
        sl = slice(i * BPC, (i + 1) * BPC)
        xt = np.ascontiguousarray(x[sl].transpose(0, 2, 1)).reshape(BPC, 2, 128, N).astype(bf)
        xst = np.ascontiguousarray(xs[sl].transpose(0, 2, 1)).reshape(BPC, 2, 128, NQ).astype(bf)
        in_maps.append({
            "xt": xt, "xst": xst,
            "wkt": wkt, "wvt": wvt, "wqt": wqt, "wpt": wpt,
            "bq": bq, "bv": bvd, "bp": bp,
            "eb": eb_d, "ones": np.ones((128, 128), ml_dtypes.bfloat16),
        })
    return in_maps


def _body(tc, a, out_ap):
    import concourse.bass as bass  # noqa: F401
    import concourse.mybir as mybir
    from contextlib import ExitStack

    nc = tc.nc
    f32 = mybir.dt.float32
    f32r = mybir.dt.float32r
    bf16 = mybir.dt.bfloat16
    AF = mybir.ActivationFunctionType
    ALU = mybir.AluOpType

    def r(ap):
        return ap

    with ExitStack() as ctx:
        ctx.enter_context(
            nc.allow_low_precision(reason="o-side bf16 is deliberate; verified vs fp32 reference")
        )
        singles = ctx.enter_context(tc.tile_pool(name="singles", bufs=1))
        # DMA order matters at startup: the first k-proj matmul only needs
        # wk + xt slice 0, so those go first (xt is issued inside the b loop)
        # wk as two separate tiles: tile-granular DMA dependencies mean the
        # first k-proj matmul (head pair 0) waits only on the first ~370ns half
        wk_a = singles.tile([128, 2, 256], bf16)
        wk_b = singles.tile([128, 2, 256], bf16)
        nc.sync.dma_start(wk_a, a["wkt"].rearrange("c p j -> p c j")[:, :, 0:256])

        def wk_sl(cc, pr):
            t = wk_a if pr < 2 else wk_b
            return t[:, cc, (pr % 2) * 128:(pr % 2) * 128 + 128]
        bqs = singles.tile([128, 4], f32)
        ones = singles.tile([128, 128], bf16)
        wq = singles.tile([128, 2, 512], bf16)
        wv = singles.tile([128, 2, 1024], bf16)
        bvs = singles.tile([128, H], f32)
        wp = singles.tile([128, 8, OUT], bf16)
        bps = singles.tile([128, OUT], f32)
        eb = singles.tile([128, H, NCH, NQ], bf16)   # exp(bias), resident all-run

        xt_p = ctx.enter_context(tc.tile_pool(name="xt", bufs=2))
        xst_p = ctx.enter_context(tc.tile_pool(name="xst", bufs=2))
        kt_p = ctx.enter_context(tc.tile_pool(name="kt", bufs=2))
        v_p = ctx.enter_context(tc.tile_pool(name="v", bufs=1))
        qt_p = ctx.enter_context(tc.tile_pool(name="qt", bufs=2))
        e_p = ctx.enter_context(tc.tile_pool(name="e", bufs=6))
        ep_p = ctx.enter_context(tc.tile_pool(name="ep", bufs=4))
        ot_p = ctx.enter_context(tc.tile_pool(name="ot", bufs=2))
        rc_p = ctx.enter_context(tc.tile_pool(name="rc", bufs=3))
        hs_p = ctx.enter_context(tc.tile_pool(name="hs", bufs=3))
        ob_p = ctx.enter_context(tc.tile_pool(name="ob", bufs=3))
        ps_work = ctx.enter_context(tc.tile_pool(name="ps_work", bufs=3, space="PSUM"))
        ps_o = ctx.enter_context(tc.tile_pool(name="ps_o", bufs=1, space="PSUM"))
        ps_sum = ctx.enter_context(tc.tile_pool(name="ps_sum", bufs=1, space="PSUM"))

        _wt_n = [0]

        def work_tile():
            _wt_n[0] += 1
            return ps_work.tile([128, GRP, 512], f32, tag="w", name=f"wt{_wt_n[0]}")

        # batch-0 startup prefetch: DMA dispatch is the startup bottleneck
        # (~650ns serial SP dispatch each), so use few, need-ordered transfers
        xt0 = xt_p.tile([128, 2, N], bf16, tag="xt0", name="xt0")
        for n0, n1 in ((0, 512), (512, 1024)):
            nc.sync.dma_start(
                xt0[:, :, n0:n1], a["xt"][0, :, :, n0:n1].rearrange("c p n -> p c n")
            )
        nc.sync.dma_start(wk_b, a["wkt"].rearrange("c p j -> p c j")[:, :, 256:512])
        nc.sync.dma_start(
            xt0[:, :, 1024:N], a["xt"][0, :, :, 1024:N].rearrange("c p n -> p c n")
        )

        for b in range(BPC):
            xt = xt0 if b == 0 else xt_p.tile([128, 2, N], bf16)
            if b > 0:
                for ns in range(3):
                    n0 = ns * 512
                    nsz = min(512, N - n0)
                    nc.sync.dma_start(
                        xt[:, :, n0:n0 + nsz],
                        a["xt"][b, :, :, n0:n0 + nsz].rearrange("c p n -> p c n"),
                    )
            xst = xst_p.tile([128, 2, NQ], bf16)
            nc.sync.dma_start(xst, a["xst"][b].rearrange("c p n -> p c n"))
            if b == 0:
                nc.sync.dma_start(wv, a["wvt"].rearrange("c p j -> p c j"))
                nc.sync.dma_start(bqs, a["bq"])
                nc.sync.dma_start(wq, a["wqt"].rearrange("c p j -> p c j"))
                nc.sync.dma_start(ones, a["ones"])
                nc.sync.dma_start(bvs, a["bv"])
                for h in range(H):
                    nc.sync.dma_start(
                        eb[:, h, :, :], a["eb"][h].rearrange("c p q -> p c q")
                    )
                nc.sync.dma_start(wp, a["wpt"].rearrange("c p j -> p c j"))
                nc.sync.dma_start(bps, a["bp"])

            kt = kt_p.tile([128, 4, N], bf16)      # [d(2 heads), pair, n]
            vt = v_p.tile([128, NCH, 1024], bf16)  # [n, chunk, v-ch head-major]
            qt = qt_p.tile([128, 4, NQ], bf16)     # [d(2 heads), pair, q]

            # --- kv/q projections ---
            # emission order k, v0-3, q, v4-9: qt lands before the first
            # head's score matmuls need it, while the late v chunks still
            # beat head 0's attn@v accumulation deadline
            def k_part(pr):
                # two 512-slices share one 2-bank psum tile -> one big copy
                ps2 = work_tile()
                for ns in range(2):
                    n0 = ns * 512
                    for cc in range(2):
                        nc.tensor.matmul(
                            ps2[:, ns, :],
                            lhsT=wk_sl(cc, pr),
                            rhs=r(xt[:, cc, n0:n0 + 512]),
                            start=(cc == 0), stop=(cc == 1),
                        )
                ps = work_tile()[:, 0, :]
                for cc in range(2):
                    nc.tensor.matmul(
                        ps[:, :256],
                        lhsT=wk_sl(cc, pr),
                        rhs=r(xt[:, cc, 1024:N]),
                        start=(cc == 0), stop=(cc == 1),
                    )
                if pr != 3:
                    nc.scalar.copy(kt[:, pr, 0:1024], ps2.rearrange("p a b -> p (a b)"))
                    nc.scalar.copy(kt[:, pr, 1024:N], ps[:, :256])
                else:
                    nc.vector.tensor_copy(kt[:, pr, 0:1024], ps2.rearrange("p a b -> p (a b)"))
                    nc.vector.tensor_copy(kt[:, pr, 1024:N], ps[:, :256])

            def v_part(cn):
                ps2 = work_tile()
                for hf in range(2):
                    for cc in range(2):
                        nc.tensor.matmul(
                            ps2[:, hf, :],
                            lhsT=r(xt[:, cc, cn * 128:(cn + 1) * 128]),
                            rhs=r(wv[:, cc, hf * 512:(hf + 1) * 512]),
                            start=(cc == 0), stop=(cc == 1),
                        )
                if cn % 2 == 0:
                    nc.scalar.copy(vt[:, cn, :], ps2.rearrange("p a b -> p (a b)"))
                else:
                    nc.vector.tensor_copy(vt[:, cn, :], ps2.rearrange("p a b -> p (a b)"))

            def q_part(pr):
                ps = work_tile()[:, 0, :]
                for cc in range(2):
                    nc.tensor.matmul(
                        ps[:, :NQ],
                        lhsT=r(wq[:, cc, pr * 128:(pr + 1) * 128]),
                        rhs=r(xst[:, cc, :]),
                        start=(cc == 0), stop=(cc == 1),
                    )
                nc.scalar.activation(
                    qt[:, pr, :], ps[:, :NQ], AF.Identity, bias=bqs[:, pr:pr + 1],
                )

            for pr in range(4):
                k_part(pr)
            for cn in range(6):
                v_part(cn)
            for pr in range(4):
                q_part(pr)
            # v chunks 6-9 are interleaved into head 0's group stream below:
            # they give the PE independent filler work right where it would
            # otherwise stall on the 3-tile exp lookahead, and their copies
            # still land before head 0's attn@v reaches chunks 6-9
            late_v = {0: (6, 7), 1: (8,), 2: (9,)}

            # --- attention per head ---
            ot = ot_p.tile([128, H, NQ], bf16)  # [d, head, q]
            if b % 2 == 0:
                # proj lhsT for a batch PAIR: q=640 = 5x128 exact (vs 2.5x128)
                t2 = hs_p.tile([128, H, 2, NQ], bf16, tag="t2", name=f"t2_{b}")
            t = t2[:, :, b % 2, :]
            for h in range(H):
                pr, p0 = h // 2, 64 * (h % 2)
                po = ps_o.tile([128, NQ], f32)
                psm = ps_sum.tile([128, NQ], f32)
                for g in range(NCH // GRP):
                    sg = work_tile()
                    for j in range(GRP):
                        c = GRP * g + j
                        nc.tensor.matmul(
                            sg[:, j, :NQ],
                            lhsT=r(kt[p0:p0 + 64, pr, c * 128:(c + 1) * 128]),
                            rhs=r(qt[p0:p0 + 64, pr, :]),
                            start=True, stop=True,
                        )
                    e = e_p.tile([128, GRP, NQ], bf16)
                    nc.scalar.activation(e, sg[:, :, :NQ], AF.Exp)
                    # relative-position bias: exp(s+b) = exp(s)*exp(b); bf16
                    # SBUF multiply runs in DVE 2x mode (vs f32 PSUM add)
                    nc.vector.tensor_tensor(
                        e, e, eb[:, h, GRP * g:GRP * (g + 1), :], ALU.mult,
                    )
                    # softmax denominator: pre-add the chunk pair on DVE so the
                    # PE streams one ones-matmul per pair; the LAST group goes
                    # direct to the PE so recip isn't gated on the DVE add
                    if 0 < g < NCH // GRP - 1:
                        epair = ep_p.tile([128, NQ], bf16)
                        nc.vector.tensor_tensor(epair, e[:, 0, :], e[:, 1, :], ALU.add)
                        nc.tensor.matmul(
                            psm,
                            lhsT=r(ones),
                            rhs=r(epair),
                            start=False, stop=False,
                        )
                    else:
                        # first and last group go direct to the PE: the first
                        # isn't gated on the DVE pair-add, and the last lets
                        # recip start without waiting for one
                        for j in range(GRP):
                            nc.tensor.matmul(
                                psm,
                                lhsT=r(ones),
                                rhs=r(e[:, j, :]),
                                start=(g == 0 and j == 0),
                                stop=(g == NCH // GRP - 1 and j == GRP - 1),
                            )
                    for j in range(GRP):
                        c = GRP * g + j
                        nc.tensor.matmul(
                            po,
                            lhsT=r(vt[:, c, h * 128:(h + 1) * 128]),
                            rhs=r(e[:, j, :]),
                            start=(c == 0), stop=(c == NCH - 1),
                        )
                    if h == 0:
                        for cn in late_v.get(g, ()):
                            v_part(cn)
                rc = rc_p.tile([128, NQ], bf16)
                nc.vector.reciprocal(rc, psm)
                oh = ot[:, h, :]
                # o = po*(1/sums) with po read straight from PSUM (no copy);
                # bv folds out of A-v (softmax rows sum to 1) and is
                # re-applied inside the hardswish below via AP-scalar operands
                nc.vector.tensor_tensor(oh, po, rc, ALU.mult)
                nc.vector.tensor_scalar_add(oh, oh, bvs[:, h:h + 1])
                th = t[:, h, :]
                # hardswish on the (otherwise idle) Pool engine; its latency
                # is hidden EXCEPT for the pair's last heads, whose t2 slices
                # the output projection needs right away -> those go on the
                # (by then idle) DVE's much faster queue
                eng = nc.vector if (b % 2 == 1 and h >= 6) else nc.gpsimd
                eng.tensor_scalar(th, oh, 3.0, 6.0, ALU.add, ALU.min)
                eng.tensor_scalar(th, th, 0.0, None, ALU.max)
                eng.tensor_tensor(th, th, oh, ALU.mult)

            # --- output projection (per batch pair, q merged to 640) ---
            if b % 2 == 1:
                out_flat = out_ap.rearrange("b q o -> (b q) o")
                for qc in range(5):
                    r0 = (b - 1) * NQ + qc * 128
                    ps = work_tile()[:, 0, :]
                    for dc in range(8):
                        nc.tensor.matmul(
                            ps[:, :OUT],
                            lhsT=t2[:, dc, :, :].rearrange(
                                "p bb q -> p (bb q)")[:, qc * 128:(qc + 1) * 128],
                            rhs=r(wp[:, dc, :]),
                            start=(dc == 0), stop=(dc == 7),
                        )
                    ob = ob_p.tile([128, OUT], bf16)
                    nc.vector.tensor_tensor(ob, ps[:, :OUT], bps, ALU.add)
                    nc.sync.dma_start(out_flat[r0:r0 + 128, :], ob)


def build():
    import concourse.mybir as mybir
    import concourse.tile as tile
    from concourse import bacc

    nc = bacc.Bacc("TRN2", target_bir_lowering=False, debug=False)
    f32, bf16 = mybir.dt.float32, mybir.dt.bfloat16
    a = {}

    def din(name, shape, dt=f32):
        a[name] = nc.dram_tensor(name, shape, dt, kind="ExternalInput").ap()

    din("xt", [BPC, 2, 128, N], bf16)
    din("xst", [BPC, 2, 128, NQ], bf16)
    din("wkt", [2, 128, 512], bf16)
    din("wvt", [2, 128, 1024], bf16)
    din("wqt", [2, 128, 512], bf16)
    din("wpt", [8, 128, OUT], bf16)
    din("bq", [128, 4])
    din("bv", [128, H])
    din("bp", [128, OUT])
    din("eb", [H, NCH, 128, NQ], bf16)
    din("ones", [128, 128], bf16)
    out_ap = nc.dram_tensor("out", [BPC, NQ, OUT], bf16, kind="ExternalOutput").ap()

    with tile.TileContext(nc) as tc:
        _body(tc, a, out_ap)
    nc.compile()
    return nc


_NC_CACHE = None


def _get_nc():
    global _NC_CACHE
    if _NC_CACHE is None:
        _NC_CACHE = build()
    return _NC_CACHE


def kernel(**inputs):
    from concourse.bass_utils import run_bass_kernel_spmd

    in_maps = _prep(inputs)
    nc = _get_nc()
    res = run_bass_kernel_spmd(nc, in_maps, list(range(NCORES)))
    out = np.concatenate([res.results[i]["out"] for i in range(NCORES)], axis=0)
    return np.ascontiguousarray(out, dtype=np.float32)


if __name__ == "__main__":
    rng = np.random.default_rng(0)
    print("smoke: building bass module...")
    nc = build()
    print("built ok:", sum(len(bb.instructions) for bb in nc.m.functions[0].blocks), "instructions")
